# revision 1
# baseline (speedup 1.0000x reference)
"""Conv1d [16,512,4096] (x) * [512,512,5] (weight) + [512] (bias) -> [16,512,4096].

Strategy: data-parallel over batch across 8 NeuronCores (2 batches/core),
weight/bias replicated. Per core the conv is computed as 5 shifted matmuls
accumulated in PSUM:

  y[o, t] = bias[o] + sum_{k=0..4} sum_{c} wprep[k, c, o] * xpad[c, t + k]

with wprep[k, c, o] = weight[o, c, 4-k] (tap-flipped, transposed on host) and
xpad = x zero-padded by 2 along t. On the PE each out tile [128 o, 512 t]
accumulates 20 matmuls (4 c-chunks x 5 taps), lhsT = wprep chunk [128 c, 128 o]
stationary, rhs = shifted x slice [128 c, 512 t] moving, dtype float32r
(1 cycle/row at free-dim >= 256 vs 4 cycles/row for plain fp32).
"""

import numpy as np

B, C, O, T, K = 16, 512, 512, 4096, 5
PAD = 2
N_CORES = 8
BPC = B // N_CORES  # batches per core
CCH = C // 128      # c chunks
OCH = O // 128      # o chunks
TT = 512            # t tile (free dim; PSUM-bank/fp32-moving max)
NTT = T // TT       # t tiles per batch
NKC = K * CCH       # accumulating matmuls per out tile

_cached = {}

# Matmul operand dtype: "f32r" (TF32-like, rel err ~1.5e-4) or "bf16"
# (rel err ~2.4e-3, faster weight loads + half the input DMA bytes).
MM_DTYPE = "f32r"


def _build_nc():
    import concourse.bacc as bacc
    import concourse.bass as bass
    import concourse.mybir as mybir
    import concourse.tile as tile

    f32 = mybir.dt.float32
    f32r = mybir.dt.float32r if MM_DTYPE == "f32r" else mybir.dt.bfloat16

    nc = bacc.Bacc(None, target_bir_lowering=False, debug=False)

    SEG = 2 * TT + 2 * PAD  # x segment width: two t-tiles + halo
    XCOLS = (NTT // 2) * 2 * TT + 2 * PAD + 4  # padded x width (4104)

    x_dram = nc.dram_tensor("x", [BPC, C, XCOLS], f32r, kind="ExternalInput")
    # host layout: [k*CCH+cc, 128c, o]
    w_dram = nc.dram_tensor("w", [NKC, 128, O], f32r, kind="ExternalInput")
    b_dram = nc.dram_tensor("b", [128, OCH], f32, kind="ExternalInput")
    y_dram = nc.dram_tensor("y", [BPC, O, T], f32, kind="ExternalOutput")

    with tile.TileContext(nc) as tc:
        with (
            tc.tile_pool(name="wp", bufs=1) as wp,
            tc.tile_pool(name="bp", bufs=1) as bp,
            tc.tile_pool(name="xp", bufs=16 + 8) as xp,
            tc.tile_pool(name="pp", bufs=8, space=bass.MemorySpace.PSUM) as pp,
            tc.tile_pool(name="op", bufs=8) as op,
        ):
            # Two HWDGE queues (ACT + SP) fill in parallel against the HBM
            # roofline. The first out-tile consumes w chunks 0..19 in order,
            # so w is split: chunks 0-9 on ACT (start immediately), chunks
            # 10-19 on SP right after the first x segments they overlap
            # with. Output stores ride ACT (w is done before they start).
            w_all = wp.tile([128, NKC * O], f32r)
            bias_sb = bp.tile([128, OCH], f32)

            def load_w(i, eng):
                eng.dma_start(w_all[:, i * O:(i + 1) * O], w_dram[i])

            for i in range(NKC // 2):
                load_w(i, nc.scalar)

            seg = {}

            def load_x(b, j2, cc):
                xt = xp.tile([128, SEG], f32r, tag="xs")
                nc.sync.dma_start(
                    xt[:],
                    x_dram[b, cc * 128:(cc + 1) * 128,
                           j2 * 2 * TT:j2 * 2 * TT + SEG],
                )
                seg[(b, cc, j2)] = xt

            for cc in range(CCH):
                load_x(0, 0, cc)
            for i in range(NKC // 2, NKC):
                load_w(i, nc.sync)
            nc.scalar.dma_start(bias_sb[:], b_dram[:])

            for b in range(BPC):
                for j2 in range(NTT // 2):
                    for cc in range(CCH):
                        if (b, j2) != (0, 0):
                            load_x(b, j2, cc)

                for oc in range(OCH):
                    for j in range(NTT):
                        ps = pp.tile([128, TT], f32)
                        # accumulate in w-chunk DMA-arrival order so the very
                        # first out-tile's matmuls pipeline with the w loads
                        for ch in range(NKC):
                            k, cc = divmod(ch, CCH)
                            lhsT = w_all[:, ch * O + oc * 128:
                                         ch * O + oc * 128 + 128]
                            off = (j % 2) * TT + k
                            rhs = seg[(b, cc, j // 2)][:, off:off + TT]
                            nc.tensor.matmul(
                                ps[:], lhsT, rhs,
                                start=(ch == 0), stop=(ch == NKC - 1),
                            )
                        ot = op.tile([128, TT], f32)
                        nc.vector.tensor_scalar_add(
                            ot[:], ps[:], bias_sb[:, oc:oc + 1]
                        )
                        nc.scalar.dma_start(
                            y_dram[b, oc * 128:(oc + 1) * 128, j * TT:(j + 1) * TT],
                            ot[:],
                        )

    nc.finalize()
    return nc


def _get_nc():
    if "nc" not in _cached:
        _cached["nc"] = _build_nc()
    return _cached["nc"]


def run(x, weight, bias, trace=False):
    from concourse.bass_utils import run_bass_kernel_spmd

    nc = _get_nc()

    x = np.asarray(x, dtype=np.float32)
    weight = np.asarray(weight, dtype=np.float32)
    bias = np.asarray(bias, dtype=np.float32)

    # Zero halo: PAD cols left, PAD+4 right (rounds segment grid to 4104
    # cols), so the kernel needs no memsets.
    x = np.pad(x, ((0, 0), (0, 0), (PAD, PAD + 4)))
    if MM_DTYPE == "bf16":
        import ml_dtypes

        x = x.astype(ml_dtypes.bfloat16)

    # wprep[k, c, o] = weight[o, c, K-1-k]; chunked over c to [K*CCH, 128, O]
    wprep = np.ascontiguousarray(
        np.flip(weight, -1).transpose(2, 1, 0).reshape(NKC, 128, O)
    )
    if MM_DTYPE == "bf16":
        import ml_dtypes

        wprep = wprep.astype(ml_dtypes.bfloat16)
    bprep = np.ascontiguousarray(bias.reshape(OCH, 128).T)  # [128, OCH]

    in_maps = [
        {"x": x[i * BPC:(i + 1) * BPC], "w": wprep, "b": bprep}
        for i in range(N_CORES)
    ]
    res = run_bass_kernel_spmd(nc, in_maps, list(range(N_CORES)), trace=trace)
    y = np.concatenate([r["y"] for r in res.results], axis=0)
    return y, res


def kernel(x, weight, bias):
    y, _ = run(x, weight, bias)
    return y



# revision 2
# speedup vs baseline: 1.0018x; 1.0018x over previous
"""Conv1d [16,512,4096] (x) * [512,512,5] (weight) + [512] (bias) -> [16,512,4096].

Current best (bf16, v5): data-parallel over batch (2 batches/core), conv as 5 shifted
matmuls accumulated in PSUM.
  - bf16 operands: halves w+x DMA bytes, rel err ~2.4e-3 (threshold 2e-2).
  - groups 0..6: chunk-outer/j-inner over 8 PSUM banks, with taps (k) inner
    and input-channel chunks (cc) outer so each x row is consumed for ~9us
    before the next is needed -> the startup DMA stays ahead.
  - last group: tile-sequential so drains spread out and the tail is short.
  - HWDGE queues only (scalar/sync): gpsimd DMA is the slow software-DGE
    path (~4x slower; it caused v4's startup gap and late-store tail).
    Loads and stores are laid out in deadline order across the two queues.
"""

import numpy as np

B, C, O, T, K = 16, 512, 512, 4096, 5
PAD = 2
N_CORES = 8
BPC = B // N_CORES  # batches per core
CCH = C // 128      # c chunks
OCH = O // 128      # o chunks
TT = 512            # t tile (free dim; PSUM-bank max for f32 out)
NTT = T // TT       # t tiles per batch
NKC = K * CCH       # accumulating matmuls per out tile
NJ2 = NTT // 2      # x segments per (b, cc)
SEG = 2 * TT + 2 * PAD           # x segment width (1028)
XCOLS = NJ2 * 2 * TT + 2 * PAD + 4  # padded x width (4104)
XH = XCOLS // 2                  # half-row split point (2052)

_cached = {}


def _build_nc():
    import concourse.bacc as bacc
    import concourse.bass as bass
    import concourse.mybir as mybir
    import concourse.tile as tile

    f32 = mybir.dt.float32
    bf16 = mybir.dt.bfloat16

    nc = bacc.Bacc(None, target_bir_lowering=False, debug=False)

    x_dram = nc.dram_tensor("x", [BPC, C, XCOLS], bf16, kind="ExternalInput")
    # host layout: [k*CCH+cc, 128c, o]
    w_dram = nc.dram_tensor("w", [NKC, 128, O], bf16, kind="ExternalInput")
    b_dram = nc.dram_tensor("b", [128, OCH], f32, kind="ExternalInput")
    y_dram = nc.dram_tensor("y", [BPC, O, T], f32, kind="ExternalOutput")

    with tile.TileContext(nc) as tc:
        with (
            tc.tile_pool(name="wp", bufs=1) as wp,
            tc.tile_pool(name="bp", bufs=1) as bp,
            tc.tile_pool(name="xp", bufs=8) as xp,
            tc.tile_pool(name="pp", bufs=8, space=bass.MemorySpace.PSUM) as pp,
            tc.tile_pool(name="op", bufs=8) as op,
        ):
            w_all = wp.tile([128, NKC * O], bf16)
            bias_sb = bp.tile([128, OCH], f32)

            xrow = {}

            def load_xrow(b, cc, eng0, eng1):
                xt = xp.tile([128, XCOLS], bf16, tag="xs")
                eng0.dma_start(xt[:, :XH],
                               x_dram[b, cc * 128:(cc + 1) * 128, :XH])
                eng1.dma_start(xt[:, XH:],
                               x_dram[b, cc * 128:(cc + 1) * 128, XH:])
                xrow[(b, cc)] = xt

            def rhs_of(b, cc, j, k):
                return xrow[(b, cc)][:, j * TT + k: j * TT + k + TT]

            def load_w(i, eng):
                eng.dma_start(w_all[:, i * O:(i + 1) * O], w_dram[i])

            # Startup in consumption-deadline order. cc row r is first used
            # at ~(start + r*8.9us); w chunk (cc,k) at ~(start +
            # (5cc+k)*1.78us). sync carries most x halves; scalar carries w
            # interleaved with the other x halves.
            load_w(0, nc.scalar)                      # needed first

            def load_xhalf(b, cc, half, eng):
                if (b, cc) not in xrow:
                    xt = xp.tile([128, XCOLS], bf16, tag="xs")
                    xrow[(b, cc)] = xt
                xt = xrow[(b, cc)]
                if half == 0:
                    eng.dma_start(xt[:, :XH],
                                  x_dram[b, cc * 128:(cc + 1) * 128, :XH])
                else:
                    eng.dma_start(xt[:, XH:],
                                  x_dram[b, cc * 128:(cc + 1) * 128, XH:])

            load_xhalf(0, 0, 0, nc.sync)
            load_xhalf(0, 0, 1, nc.scalar)
            for k in range(1, K):                     # w (cc0, k1..k4)
                load_w(k * CCH, nc.scalar)
            load_xhalf(0, 1, 1, nc.sync)
            load_xhalf(0, 1, 0, nc.scalar)
            for k in range(K):                        # w (cc1, *)
                load_w(k * CCH + 1, nc.scalar)
            load_xhalf(0, 2, 0, nc.sync)
            load_xhalf(0, 2, 1, nc.scalar)
            for k in range(K):                        # w (cc2, *)
                load_w(k * CCH + 2, nc.scalar)
            load_xhalf(0, 3, 1, nc.sync)
            load_xhalf(0, 3, 0, nc.scalar)
            for k in range(K):                        # w (cc3, *)
                load_w(k * CCH + 3, nc.scalar)
            nc.scalar.dma_start(bias_sb[:], b_dram[:])

            def drain(b, oc, j, ps):
                ot = op.tile([128, TT], f32, name="ot")
                nc.vector.tensor_scalar_add(
                    ot[:], ps[:], bias_sb[:, oc:oc + 1])
                eng = nc.scalar if j % 2 else nc.sync
                eng.dma_start(
                    y_dram[b, oc * 128:(oc + 1) * 128, j * TT:(j + 1) * TT],
                    ot[:])

            def lhsT_of(oc, ch):
                return w_all[:, ch * O + oc * 128: ch * O + oc * 128 + 128]

            groups = [(b, oc) for b in range(BPC) for oc in range(OCH)]
            for gi, (b, oc) in enumerate(groups):
                if gi == 2:
                    # prefetch next batch mid-way through this one
                    for cc in range(CCH):
                        load_xrow(1, cc,
                                  nc.scalar if cc % 2 else nc.sync,
                                  nc.sync if cc % 2 else nc.scalar)
                # cc-outer / k-inner: each x row is used for 5 consecutive
                # chunks before the next row is needed
                chunks = [(k * CCH + cc, cc, k)
                          for cc in range(CCH) for k in range(K)]
                if gi < len(groups) - 1:
                    ps = [pp.tile([128, TT], f32, name="ps")
                          for _ in range(NTT)]
                    for ci, (ch, cc, k) in enumerate(chunks):
                        for j in range(NTT):
                            nc.tensor.matmul(
                                ps[j][:], lhsT_of(oc, ch), rhs_of(b, cc, j, k),
                                start=(ci == 0), stop=(ci == NKC - 1))
                    for j in range(NTT):
                        drain(b, oc, j, ps[j])
                else:
                    # last group: tile-sequential, drain as soon as each
                    # tile stops
                    for j in range(NTT):
                        ps_t = pp.tile([128, TT], f32, name="ps")
                        for ci, (ch, cc, k) in enumerate(chunks):
                            nc.tensor.matmul(
                                ps_t[:], lhsT_of(oc, ch), rhs_of(b, cc, j, k),
                                start=(ci == 0), stop=(ci == NKC - 1))
                        drain(b, oc, j, ps_t)

    nc.finalize()
    return nc


def _get_nc():
    if "nc" not in _cached:
        _cached["nc"] = _build_nc()
    return _cached["nc"]


def run(x, weight, bias, trace=False):
    import ml_dtypes
    from concourse.bass_utils import run_bass_kernel_spmd

    nc = _get_nc()

    x = np.asarray(x, dtype=np.float32)
    weight = np.asarray(weight, dtype=np.float32)
    bias = np.asarray(bias, dtype=np.float32)

    # Zero halo: PAD cols left, PAD+4 right -> width 4104, so the kernel
    # needs no memsets.
    x = np.pad(x, ((0, 0), (0, 0), (PAD, PAD + 4))).astype(ml_dtypes.bfloat16)
    # wprep[k, c, o] = weight[o, c, K-1-k]; chunked over c to [K*CCH, 128, O]
    wprep = np.ascontiguousarray(
        np.flip(weight, -1).transpose(2, 1, 0).reshape(NKC, 128, O)
    ).astype(ml_dtypes.bfloat16)
    bprep = np.ascontiguousarray(bias.reshape(OCH, 128).T)  # [128, OCH]

    in_maps = [
        {"x": x[i * BPC:(i + 1) * BPC], "w": wprep, "b": bprep}
        for i in range(N_CORES)
    ]
    res = run_bass_kernel_spmd(nc, in_maps, list(range(N_CORES)), trace=trace)
    y = np.concatenate([r["y"] for r in res.results], axis=0)
    return y, res


def kernel(x, weight, bias):
    y, _ = run(x, weight, bias)
    return y
